# revision 13
# baseline (speedup 1.0000x reference)
"""Bayesian linear layer on 8 Trainium2 NeuronCores.

Computes: weight = mu + softplus(rho) * eps  (elementwise, [O, I])
          bias   = b_mu + softplus(b_rho) * b_eps              ([O])
          y      = x @ weight.T + bias       ([N, I] @ [I, O] -> [N, O])

Shapes: x [8192, 4096], weight_* [16384, 4096], bias_* [16384].

Sharding: column-parallel over 8 cores — each core owns 2048 output
features (its slice of the weight/bias params), x is replicated. Each
core computes an independent [8192, 2048] output slice; the host
concatenates along the feature dim. No collectives needed.

Device kernel (SPMD, one Bass program, per-core data):
 - softplus computed as Ln(Exp(rho) + 1) on the scalar engine (no
   softplus entry in this arch's act tables); the *eps +mu chain
   alternates between DVE and GpSimd so PSUM evictions on DVE are
   never stuck behind a materialization backlog.
 - weights materialized on-chip into resident bf16 SBUF tiles
   [128 i-part, 2048 o] x 32 k-tiles (128 KB/partition).
 - x pre-tiled on host to [128 part, mc, kt, m] bf16 so each chunk
   load is 128 x 16KB fully-contiguous descriptors (a strided gather
   costs ~7us of HWDGE descriptor-generation per chunk and starves
   the PE).
 - startup "chase": a k-outer sweep over the first 4 x-chunks tracks
   the block-0 weight-materialization wavefront (chunks join the sweep
   as their x tiles land; earlier k's are backfilled into PSUM, which
   is order-independent), then a second sweep does block 1. This keeps
   the PE ~fully busy while the first ~25 MB of params stream in.
 - steady state: per chunk, one k-outer loop accumulates both j-blocks
   (2 PSUM tiles) and evicts with a bias-add DVE pass (bf16 out, host
   upcasts), y writebacks on the Scalar HWDGE queue, x loads on Sync.
"""

import numpy as np
import ml_dtypes

import concourse.bass as bass
import concourse.mybir as mybir
import concourse.tile as tile
from concourse.bass_utils import run_bass_kernel_spmd
from concourse.vector_clock import ScopedClock, VectorClock

N_CORES = 8
N_TOK = 8192
IN_F = 4096
OUT_F = 16384
O_PER = OUT_F // N_CORES  # 2048 out features per core

P = 128
KT = IN_F // P       # 32 k-tiles
OC = 512             # o-chunk for weight materialization + matmul N
NOC = O_PER // OC    # 4 o-chunks

F32 = mybir.dt.float32
BF16 = mybir.dt.bfloat16
AF = mybir.ActivationFunctionType
ALU = mybir.AluOpType


def _patch_tile_drain():
    """The walrus build here caps sync-wait commands per CTRL_NO_STRUCT
    instruction; Tile's kernel-tail Drain overflows it. Spread the waits
    across nop carriers (one wait each) before the drain."""
    if getattr(tile.TileContext, "_drain_patched", False):
        return

    def _drain_and_barrier(self, tick_clock, wait_clock):
        nc = self.nc
        gc = tick_clock.global_clock
        n = len(gc)
        for i in range(n):
            t = gc[i]
            if t > 0:
                sub = [0] * n
                sub[i] = t
                carrier = nc.sync.nop(nofuse=True)
                wait_clock.add_sem_waits(
                    carrier.ins, ScopedClock({None: VectorClock(sub)})
                )
        nc.sync.drain()
        nc.all_engine_barrier()
        popped = nc._tile_sem_poison_stack.pop()
        assert popped is self._sem_poison
        nc.clear_and_free_semaphores(list(self.sems.allocated().values()))
        nc.all_engine_barrier()

    tile.TileContext._drain_and_barrier = _drain_and_barrier
    tile.TileContext._drain_patched = True


def _split_sync_waits(nc, max_waits=1):
    """This container's walrus build accepts at most ONE sync-wait command
    per instruction (a 2-wait TensorTensor fails codegen with 'Too many
    sync wait commands'). Tile emits up to 3. Spill the excess onto
    same-engine InstNoOp carriers inserted immediately before the
    overloaded instruction — same-engine program order preserves the
    wait-before-execute semantics."""
    n_spilled = 0
    for fn in nc.m.functions:
        for bb in fn.blocks:
            insts = list(bb.instructions)
            out = []
            changed = False
            for inst in insts:
                si = inst.sync_info
                if si is not None and si.on_wait and len(si.on_wait) > max_waits:
                    waits = list(si.on_wait)
                    spill, keep = waits[:-max_waits], waits[-max_waits:]
                    for w in spill:
                        nop = mybir.InstNoOp(
                            name=f"I-waitspill-{nc.next_id()}", ins=[], outs=[]
                        )
                        nop.engine = inst.engine
                        nop.sync_info = mybir.SyncInfo(on_wait=[w], on_update=[])
                        out.append(nop)
                        n_spilled += 1
                    inst.sync_info = mybir.SyncInfo(
                        on_wait=keep, on_update=list(si.on_update)
                    )
                    changed = True
                out.append(inst)
            if changed:
                bb.instructions = out
    return n_spilled


M_CHUNK = 256            # tokens per x tile (2 lhsT subtiles of 128)
MC = N_TOK // M_CHUNK    # 32 m-chunks
MSUB = M_CHUNK // P      # 2
OCS = 512                # stage chunk for weight materialization
CHASE = 4                # x-chunks processed by the startup chase sweeps
JOIN = [0, 6, 16, 20]    # sweep-k at which chase chunk c joins (x arrival)


def _build():
    _patch_tile_drain()
    nc = bass.Bass()

    xtl = nc.dram_tensor("xtl", [P, MC, KT, M_CHUNK], BF16, kind="ExternalInput")
    wmuT = nc.dram_tensor("wmuT", [IN_F, O_PER], BF16, kind="ExternalInput")
    wrhoT = nc.dram_tensor("wrhoT", [IN_F, O_PER], BF16, kind="ExternalInput")
    wepsT = nc.dram_tensor("wepsT", [IN_F, O_PER], BF16, kind="ExternalInput")
    bmu = nc.dram_tensor("bmu", [1, O_PER], BF16, kind="ExternalInput")
    brho = nc.dram_tensor("brho", [1, O_PER], BF16, kind="ExternalInput")
    beps = nc.dram_tensor("beps", [1, O_PER], BF16, kind="ExternalInput")
    y = nc.dram_tensor("y", [N_TOK, O_PER], BF16, kind="ExternalOutput")

    with tile.TileContext(nc) as tc:
        with (
            tc.tile_pool(name="wpool", bufs=1) as wpool,
            tc.tile_pool(name="stage", bufs=3) as stage,
            tc.tile_pool(name="xpool", bufs=CHASE) as xpool,
            tc.tile_pool(name="opool", bufs=2) as opool,
            tc.tile_pool(name="bpool", bufs=1) as bpool,
            tc.tile_pool(name="psum", bufs=4, space="PSUM") as psump,
        ):
            # resident bf16 weights for all 4 o-blocks:
            # 4 x 32 x [128, 512] bf16 = 128 KB/partition
            w_tiles = {
                (j, k): wpool.tile([P, OC], BF16, name=f"w_{j}_{k}", tag=f"w_{j}_{k}")
                for j in range(NOC)
                for k in range(KT)
            }

            bias_bc = bpool.tile([P, O_PER], BF16, name="bias_bc")

            def stage_tiles():
                rho_s = stage.tile([P, OCS], BF16, name="rho_s", tag="rho_s")
                eps_s = stage.tile([P, OCS], BF16, name="eps_s", tag="eps_s")
                exp_s = stage.tile([P, OCS], BF16, name="exp_s", tag="exp_s")
                return rho_s, eps_s, exp_s

            def materialize_ktile(j, k):
                # w[j, k][:, :] = mu + softplus(rho)*eps for o-block j.
                # mu is DMA'd straight into the w tile and the final add runs
                # in place (one fewer stage tile -> deeper rotation, which
                # must outpace the ACT-bound 1.44us/ktile wavefront cadence).
                # Blocks 0/1 gate the startup chase: their loads ride the
                # Sync ring and their fma runs on the fast DVE. Blocks 2/3
                # materialize during pair-0 steady state: their loads go on
                # the GpSimd (SWDGE) ring BEHIND that chunk's y writebacks —
                # the queue's FIFO order throttles the ~25 MB param download
                # to the chunk cadence (issued greedily it floods HBM for
                # ~150us and starves the x-tile stream), and fma runs on the
                # otherwise-idle GpSimd.
                ksl = slice(k * P, (k + 1) * P)
                csl = slice(j * OC, (j + 1) * OC)
                rho_s, eps_s, exp_s = stage_tiles()
                dma = nc.sync if j < 2 else nc.gpsimd
                eng = nc.vector if j < 2 else nc.gpsimd
                dma.dma_start(w_tiles[(j, k)], wmuT[ksl, csl])
                dma.dma_start(rho_s, wrhoT[ksl, csl])
                dma.dma_start(eps_s, wepsT[ksl, csl])
                nc.scalar.activation(exp_s, rho_s, AF.Exp)
                nc.scalar.activation(exp_s, exp_s, AF.Ln, bias=1.0)
                eng.tensor_mul(exp_s, exp_s, eps_s)
                eng.tensor_add(w_tiles[(j, k)], w_tiles[(j, k)], exp_s)

            def emit_bias():
                # bias softplus fma on partition 0, then replicate to all
                # 128 partitions via a doubling SBUF->SBUF DMA ladder on the
                # GpSimd (SWDGE) queue — the serial ladder must never block
                # the Sync queue that feeds the PE.
                for oc in range(O_PER // OCS):
                    sl = bass.ts(oc, OCS)
                    rho_s, eps_s, exp_s = stage_tiles()
                    nc.gpsimd.dma_start(bias_bc[0:1, sl], bmu[0:1, sl])
                    nc.gpsimd.dma_start(rho_s[0:1, :], brho[0:1, sl])
                    nc.gpsimd.dma_start(eps_s[0:1, :], beps[0:1, sl])
                    nc.scalar.activation(exp_s[0:1, :], rho_s[0:1, :], AF.Exp)
                    nc.scalar.activation(exp_s[0:1, :], exp_s[0:1, :], AF.Ln, bias=1.0)
                    nc.vector.tensor_mul(exp_s[0:1, :], exp_s[0:1, :], eps_s[0:1, :])
                    nc.vector.tensor_add(bias_bc[0:1, sl], bias_bc[0:1, sl], exp_s[0:1, :])
                rep = 1
                while rep < P:
                    nc.gpsimd.dma_start(bias_bc[rep : 2 * rep, :], bias_bc[0:rep, :])
                    rep *= 2

            def evict(ps, mc, j):
                # PSUM -> SBUF bias-add (DVE), bf16 out, y store on the
                # GpSimd SWDGE ring — its own DMA path, never behind the
                # stage-rotation waits on the Scalar ring or x loads on Sync.
                jsl = bass.ts(j, OC)
                for s in range(MSUB):
                    out_sb = opool.tile([P, OC], BF16, name="out_sb", tag="out_sb")
                    nc.vector.scalar_tensor_tensor(
                        out_sb,
                        ps[:, bass.ts(s, OC)],
                        1.0,
                        bias_bc[:, jsl],
                        op0=ALU.bypass,
                        op1=ALU.add,
                    )
                    nc.gpsimd.dma_start(
                        y[mc * M_CHUNK + s * P : mc * M_CHUNK + (s + 1) * P, jsl],
                        out_sb,
                    )

            # ── startup DMA interleave: x chunks 0..3 slot into the block-0
            # param stream so the chase sweep can consume both as they land.
            xts = []

            def load_chase_xt(c):
                xt = xpool.tile([P, KT, M_CHUNK], BF16, name="xt", tag="xt")
                nc.sync.dma_start(xt, xtl[:, c])
                xts.append(xt)

            load_chase_xt(0)
            for k in range(0, 2):
                materialize_ktile(0, k)
            load_chase_xt(1)
            for k in range(2, 8):
                materialize_ktile(0, k)
            load_chase_xt(2)
            for k in range(8, 16):
                materialize_ktile(0, k)
            emit_bias()
            load_chase_xt(3)
            for k in range(16, KT):
                materialize_ktile(0, k)

            # ── chase sweep W1 (block 0): k-outer over 4 chunks; chunk c
            # joins at k=JOIN[c] (backfilling k<JOIN[c] first — PSUM
            # accumulation is order-independent). Emitted BEFORE block 1's
            # materialization so W1's eviction STTs sit ahead of block-1's
            # fma ops in the DVE queue — W2's PSUM handoff must not wait
            # behind them.
            def sweep_mm(ps_c, c, j, k, start, stop):
                for s in range(MSUB):
                    nc.tensor.matmul(
                        ps_c[:, bass.ts(s, OC)],
                        xts[c][:, k, bass.ts(s, P)],
                        w_tiles[(j, k)],
                        start=start,
                        stop=stop,
                    )

            pss = [
                psump.tile([P, MSUB * OC], F32, name="ps", tag="ps")
                for _ in range(CHASE)
            ]
            for k in range(KT):
                for c in range(CHASE):
                    if k == JOIN[c] and JOIN[c] > 0:
                        for kk in range(JOIN[c]):
                            sweep_mm(pss[c], c, 0, kk, start=(kk == 0), stop=False)
                    if k >= JOIN[c]:
                        sweep_mm(
                            pss[c], c, 0, k,
                            start=(k == 0 and JOIN[c] == 0),
                            stop=(k == KT - 1),
                        )
            for c in range(CHASE):
                evict(pss[c], c, 0)

            for k in range(KT):
                materialize_ktile(1, k)

            # ── chase sweep W2 (block 1): plain k-outer over the same 4
            # resident x chunks.
            pss2 = [
                psump.tile([P, MSUB * OC], F32, name="ps", tag="ps")
                for _ in range(CHASE)
            ]
            for k in range(KT):
                for c in range(CHASE):
                    sweep_mm(pss2[c], c, 1, k, start=(k == 0), stop=(k == KT - 1))
            for c in range(CHASE):
                evict(pss2[c], c, 1)

            # ── steady state: per chunk accumulate both j-blocks in one
            # k-outer loop (2 PSUM tiles), so there is one group boundary
            # per chunk instead of two.
            def steady_chunk(mc, j0, j1):
                xt = xpool.tile([P, KT, M_CHUNK], BF16, name="xt", tag="xt")
                nc.sync.dma_start(xt, xtl[:, mc])
                ps0 = psump.tile([P, MSUB * OC], F32, name="ps", tag="ps")
                ps1 = psump.tile([P, MSUB * OC], F32, name="ps", tag="ps")
                for k in range(KT):
                    for s in range(MSUB):
                        nc.tensor.matmul(
                            ps0[:, bass.ts(s, OC)],
                            xt[:, k, bass.ts(s, P)],
                            w_tiles[(j0, k)],
                            start=(k == 0),
                            stop=(k == KT - 1),
                        )
                        nc.tensor.matmul(
                            ps1[:, bass.ts(s, OC)],
                            xt[:, k, bass.ts(s, P)],
                            w_tiles[(j1, k)],
                            start=(k == 0),
                            stop=(k == KT - 1),
                        )
                evict(ps0, mc, j0)
                evict(ps1, mc, j1)

            # pair 0 steady chunks, with blocks 2/3 materialization spread
            # evenly across them
            for mc in range(CHASE, MC):
                steady_chunk(mc, 0, 1)
                i0 = (mc - CHASE) * (2 * KT) // (MC - CHASE)
                i1 = (mc - CHASE + 1) * (2 * KT) // (MC - CHASE)
                for idx in range(i0, i1):
                    materialize_ktile(2 + idx // KT, idx % KT)

            # pair 1: all chunks on blocks {2, 3}
            for mc in range(MC):
                steady_chunk(mc, 2, 3)

    _split_sync_waits(nc)
    nc.finalize()
    return nc


_NC_CACHE = None


def _get_nc():
    global _NC_CACHE
    if _NC_CACHE is None:
        _NC_CACHE = _build()
    return _NC_CACHE


def prepare_in_maps(x, weight_mu, weight_rho, weight_eps, bias_mu, bias_rho, bias_eps):
    x = np.asarray(x, dtype=np.float32)
    weight_mu = np.asarray(weight_mu, dtype=np.float32)
    weight_rho = np.asarray(weight_rho, dtype=np.float32)
    weight_eps = np.asarray(weight_eps, dtype=np.float32)
    bias_mu = np.asarray(bias_mu, dtype=np.float32)
    bias_rho = np.asarray(bias_rho, dtype=np.float32)
    bias_eps = np.asarray(bias_eps, dtype=np.float32)

    # xtl[p, mc, kt, m] = x[mc*M_CHUNK + m, kt*P + p] — per-(p, mc) the
    # [kt, m] block is contiguous, so a chunk load is 128 x 16KB descriptors.
    xtl = np.ascontiguousarray(
        x.reshape(MC, M_CHUNK, KT, P).transpose(3, 0, 2, 1)
    ).astype(ml_dtypes.bfloat16)
    in_maps = []
    for c in range(N_CORES):
        osl = slice(c * O_PER, (c + 1) * O_PER)
        in_maps.append(
            {
                "xtl": xtl,
                "wmuT": np.ascontiguousarray(weight_mu[osl, :].T).astype(ml_dtypes.bfloat16),
                "wrhoT": np.ascontiguousarray(weight_rho[osl, :].T).astype(ml_dtypes.bfloat16),
                "wepsT": np.ascontiguousarray(weight_eps[osl, :].T).astype(ml_dtypes.bfloat16),
                "bmu": bias_mu[osl].reshape(1, O_PER).astype(ml_dtypes.bfloat16),
                "brho": bias_rho[osl].reshape(1, O_PER).astype(ml_dtypes.bfloat16),
                "beps": bias_eps[osl].reshape(1, O_PER).astype(ml_dtypes.bfloat16),
            }
        )
    return in_maps


def run(in_maps, trace=False):
    nc = _get_nc()
    res = run_bass_kernel_spmd(nc, in_maps, list(range(N_CORES)), trace=trace)
    out = np.concatenate(
        [res.results[c]["y"].astype(np.float32) for c in range(N_CORES)], axis=1
    )
    return out, res


def kernel(**inputs) -> np.ndarray:
    in_maps = prepare_in_maps(**inputs)
    out, _ = run(in_maps, trace=False)
    return out
